# revision 17
# baseline (speedup 1.0000x reference)
"""Trainium2 Bass kernel for nn_CompNet (spiking LIF RNN) — V2.

Strategy vs. the V1 baseline (191 us):
  * Time-axis speculative parallelism: T=250 steps split into 8 slices of 32
    (T padded to 256); each core runs its slice plus a W=12-step warmup from
    zero state.  LIF state decays 2^-1 per step and hard-resets, so the
    trajectory reconverges exactly within 12 steps (validated in numpy:
    0 spike flips at W=16, <10 harmless flips at W=12).  Serial-loop length
    drops 250 -> 44 per core; every core carries the full batch B=256.
  * Resident PE weights via tile_position (no per-step LDWEIGHTS):
      rows 0-63   cols 0-83 : L      (recurrent + readout, loaded once)
      rows 64-127 cols 0-63 : I64    (E-injection identity, loaded once)
      row  64     cols 64-83: B20    (LIF2 constant row, reloaded after
                                      big-mm bursts that clobber cols 64-127)
      rows 0-127  cols 64-127: big-mm weights (fp8 DoubleRow, rotating)
    Loop matmuls carry ins.ldweights=False so the stationary operand stays.
  * Per step j: PE  ps = E_j-inject + B20-inject + L@M_j   (3 MMs, only the
    L@M one waits on the mask); DVE  mask/u/cu (bf16 state); GPSIMD
    accumulates LIF2 (and LIF1, as a self-check channel) mask counts into
    three j-range buckets so the host can drop warmup/out-of-range steps
    per core without breaking SPMD uniformity.
  * Feedforward drive E = Wtil@x (+bias) in fp8 DoubleRow (wtil scaled by 64
    host-side, un-scaled in the ACT evacuation), interleaved with the loop.
    fp8 x halves DMA to ~8.7 MB/core; numpy-validated: v2 margin to
    threshold stays ~0.45, output unchanged.

Math (same negated encoding as V1):
  qsum_j = (1-bt) - 0.5*Wrh@W1@x_j - 0.5*[Wry;W2]@m_{j-1};  m = (v < 1)
  spike test: v_j < 1  <=>  0.5*cu_{j-1} < qsum_j
  u = 0.5*cu - qsum = v - 1;  cu' = (u+1)*m
Stats: sum of m2 (rows 64-83) over the valid window; out = (235 - S)/235.
"""

import numpy as np
import ml_dtypes

BF16 = ml_dtypes.bfloat16
E4 = ml_dtypes.float8_e4m3

B, T, D, H, C = 256, 250, 700, 64, 20
NCORES = 8
S_SPLIT = 8               # time slices
SEG = 32                  # steps per slice (T padded to 256)
W = 10                    # warmup steps
N = SEG + W               # 44 real steps per core
NITER = N + 1             # +1 loop iter for the lagged LIF2 mask tail
BL = B                    # 256 batch columns per step block
NCOL = N * BL             # 11264 drive columns per core
P = H + C                 # 84 stacked rows (LIF1 + LIF2)
DP = 768                  # padded feature dim (3 fp8-DR chunks of 256)
WSCALE = 64.0             # fp8 weight pre-scale (undone in ACT evacuation)
VTH_INIT = 2.0e9          # suppresses the phantom LIF2 step at j=0

# stats buckets (block index = local step + 2)
BLK_LO, BLK_MID0, BLK_MID1, BLK_HI = W + 2, W + 17, W + 28, N + 2

USE_DR = False            # fp8-normal measured == DR throughput; simpler
USE_RESIDENT = True       # ins.ldweights=False resident-weight loop MMs

_CACHE = {}


def _build_nc():
    import concourse.bass as bass
    import concourse.mybir as mybir
    from concourse.tile import TileContext

    dt = mybir.dt
    AF = mybir.ActivationFunctionType
    OP = mybir.AluOpType
    PM = mybir.MatmulPerfMode
    ts = bass.ts

    nc = bass.Bass(
        "TRN2", target_bir_lowering=False, debug=False,
        detect_race_conditions=False,
    )

    xdt = dt.float8e4
    if USE_DR:
        xq = nc.dram_tensor("xq", [3, 128, 2, NCOL], xdt, kind="ExternalInput").ap()
        Wq = nc.dram_tensor("Wq", [3, 128, 2, H], xdt, kind="ExternalInput").ap()
    else:
        xq = nc.dram_tensor("xq", [6, 128, NCOL], xdt, kind="ExternalInput").ap()
        Wq = nc.dram_tensor("Wq", [6, 128, H], xdt, kind="ExternalInput").ap()
    Lw = nc.dram_tensor("Lw", [H, P], dt.bfloat16, kind="ExternalInput").ap()
    I64d = nc.dram_tensor("I64d", [H, H], dt.bfloat16, kind="ExternalInput").ap()
    B20d = nc.dram_tensor("B20d", [1, C], dt.bfloat16, kind="ExternalInput").ap()
    b64 = nc.dram_tensor("b64", [H, 1], dt.float32, kind="ExternalInput").ap()
    out_d = nc.dram_tensor("out", [P, 3 * BL], dt.float32, kind="ExternalOutput").ap()

    res_mms = []

    # x DMA pieces: 1024-col granularity, emitted progressively
    XP = 1024
    NXP = NCOL // XP          # 11 pieces per chunk
    NPIECE = NCOL // 512      # 22 big-mm pieces

    with TileContext(nc) as tc:
        with (
            tc.tile_pool(name="const", bufs=1) as cp,
            tc.tile_pool(name="psL", bufs=3, space="PSUM") as psL,
            tc.tile_pool(name="psF", bufs=2, space="PSUM") as psF,
            tc.tile_pool(name="wk", bufs=3) as wp,
        ):
            # ---- persistent tiles ----
            L_t = cp.tile([H, P], dt.bfloat16, tag="L")
            IF_t = cp.tile([128, H], dt.bfloat16, tag="I")
            I_t = IF_t[64:128, :]
            BF_t = cp.tile([128, C], dt.bfloat16, tag="B20")
            B20_t = BF_t[64:65, :]
            ON_t = cp.tile([128, BL], dt.bfloat16, tag="ones")
            ones_t = ON_t[64:65, :]
            EF_t = cp.tile([128, NCOL], dt.bfloat16, tag="EF")
            E_t = EF_t[64:128, :]
            M_t = cp.tile([P, (NITER + 1) * BL], dt.bfloat16, tag="M")
            cu0 = cp.tile([P, BL], dt.float32, tag="cu0")
            bb_t = cp.tile([128, 1], dt.float32, tag="bb")
            b64_t = bb_t[64:128, :]
            S_t = cp.tile([P, 3 * BL], dt.float32, tag="St")
            S_lo = S_t[:, 0:BL]
            S_md = S_t[:, BL:2 * BL]
            S_hi = S_t[:, 2 * BL:3 * BL]
            if USE_DR:
                xts = [cp.tile([128, 2, NCOL], xdt, tag=f"x{c}", name=f"xts{c}")
                       for c in range(3)]
                wts = [cp.tile([128, 2, H], xdt, tag=f"w{c}", name=f"wts{c}")
                       for c in range(3)]
            else:
                xts = [cp.tile([128, NCOL], xdt, tag=f"x{c}", name=f"xts{c}")
                       for c in range(6)]
                wts = [cp.tile([128, H], xdt, tag=f"w{c}", name=f"wts{c}")
                      for c in range(6)]

            # ---- prologue DMAs + inits ----
            for c in range(len(wts)):
                nc.sync.dma_start(out=wts[c][:], in_=Wq[c])
            nc.sync.dma_start(out=L_t[:, :], in_=Lw[:, :])
            nc.sync.dma_start(out=I_t[:, :], in_=I64d[:, :])
            nc.sync.dma_start(out=BF_t[64:65, :], in_=B20d[:, :])
            nc.sync.dma_start(out=bb_t[64:128, :], in_=b64[:, :])

            # absorb the b64 DMA wait on ACT here so the per-piece evacuation
            # activations carry only their PE wait (walrus 1-wait limit)
            btc = cp.tile([128, 1], dt.float32, tag="btc")
            nc.scalar.activation(
                out=btc[64:128, :], in_=b64_t[:, :],
                func=AF.Copy, bias=0.0, scale=1.0)

            nc.vector.memset(ON_t[64:65, :], 1.0)
            nc.vector.memset(M_t[0:H, 0:BL], 1.0)      # m_{-1}=1 (y=0)
            nc.vector.memset(M_t[H:P, 0:BL], 0.0)
            nc.vector.memset(cu0[0:H, :], 0.0)
            nc.vector.memset(cu0[H:P, :], VTH_INIT)
            nc.gpsimd.memset(S_t[:, :], 0.0)

            def emit_xdma_cols(c0, w):
                for c in range(len(xts)):
                    if USE_DR:
                        nc.sync.dma_start(out=xts[c][:, :, c0:c0 + w],
                                          in_=xq[c, :, :, c0:c0 + w])
                    else:
                        nc.sync.dma_start(out=xts[c][:, c0:c0 + w],
                                          in_=xq[c, :, c0:c0 + w])

            def emit_xdma(d):
                emit_xdma_cols(d * XP, XP)

            def emit_piece(p):
                """Big-mm piece: 512 drive columns -> E (PSUM->ACT->SBUF)."""
                c0 = p * 512
                pf = psF.tile([128, 512], dt.float32, tag="pf")
                nch = len(wts)
                for c in range(nch):
                    if USE_DR:
                        nc.tensor.matmul(
                            out=pf[64:128, :], lhsT=wts[c][:, :, :],
                            rhs=xts[c][:, :, c0:c0 + 512],
                            start=(c == 0), stop=(c == nch - 1),
                            perf_mode=PM.DoubleRow)
                    else:
                        nc.tensor.matmul(
                            out=pf[64:128, :], lhsT=wts[c][:, :],
                            rhs=xts[c][:, c0:c0 + 512],
                            start=(c == 0), stop=(c == nch - 1),
                            tile_position=(0, 64))
                nc.scalar.activation(
                    out=EF_t[64:128, c0:c0 + 512], in_=pf[64:128, :],
                    func=AF.Identity, bias=b64_t[:, 0:1], scale=1.0 / WSCALE)

            # prologue: first x pieces + big-mm pieces + resident weights
            for c0 in range(0, 2048, 256):
                emit_xdma_cols(c0, 256)
            nc.tensor.ldweights(L_t[:, :], tile_position=(0, 0))
            nc.tensor.ldweights(IF_t[64:128, :], tile_position=(64, 0))
            emit_piece(0)
            emit_piece(1)

            extras = {}
            for p in range(2, NPIECE):
                extras.setdefault(max(0, 2 * p - 4), []).append(
                    lambda p=p: emit_piece(p))
            c0 = 2048
            while c0 < NCOL:
                w = min(1024, NCOL - c0)
                it = max(0, 2 * (c0 // 512) - 10)
                extras.setdefault(it, []).append(
                    lambda c0=c0, w=w: emit_xdma_cols(c0, w))
                c0 += w

            # ---- the serial LIF loop ----
            cu_prev = cu0
            for j in range(NITER):
                for th in extras.pop(j, []):
                    th()
                ps = psL.tile([P, BL], dt.float32, tag="ps")
                ej = j if j < N else 0
                # wait discipline (walrus: one sync wait per compute inst):
                #   i2 self-loads B20 (waits: psum-bank WAR on DVE)
                #   i1 resident I64   (waits: ACT wrote E block)
                #   i3 resident L     (waits: DVE mask block j)
                # start=True on BOTH injects: has_written/pending-zero is
                # per-partition (each start covers its own out partitions),
                # i3 then accumulates across all 84.
                i2 = nc.tensor.matmul(
                    out=ps[H:P, :], lhsT=B20_t[:, :], rhs=ones_t[:, :],
                    start=True, stop=False, tile_position=(64, 64),
                    skip_group_check=True)
                i1 = nc.tensor.matmul(
                    out=ps[0:H, :], lhsT=I_t[:, :], rhs=E_t[:, ts(ej, BL)],
                    start=True, stop=False, tile_position=(64, 0),
                    skip_group_check=True)
                i3 = nc.tensor.matmul(
                    out=ps[:, :], lhsT=L_t[:, :], rhs=M_t[0:H, ts(j, BL)],
                    start=False, stop=True, tile_position=(0, 0),
                    skip_group_check=True)
                res_mms.extend([i1, i3])
                nc.vector.scalar_tensor_tensor(
                    out=M_t[:, ts(j + 1, BL)], in0=cu_prev[:, :], scalar=0.5,
                    in1=ps[:, :], op0=OP.mult, op1=OP.is_lt)
                if j < NITER - 1:
                    u = wp.tile([P, BL], dt.bfloat16, tag="u")
                    nc.vector.scalar_tensor_tensor(
                        out=u[:, :], in0=cu_prev[:, :], scalar=0.5,
                        in1=ps[:, :], op0=OP.mult, op1=OP.subtract)
                    cu = wp.tile([P, BL], dt.float32, tag="cu")
                    nc.vector.scalar_tensor_tensor(
                        out=cu[:, :], in0=u[:, :], scalar=1.0,
                        in1=M_t[:, ts(j + 1, BL)], op0=OP.add, op1=OP.mult)
                    cu_prev = cu
                # stats: mask block j+1 into its j-range bucket (full 84 rows;
                # rows 0-63 double as a host-side self-check channel)
                blk = j + 1
                if BLK_LO <= blk < BLK_MID0:
                    nc.gpsimd.tensor_tensor(
                        out=S_lo[:, :], in0=S_lo[:, :],
                        in1=M_t[:, ts(blk, BL)], op=OP.add)
                elif BLK_MID0 <= blk < BLK_MID1:
                    nc.gpsimd.tensor_tensor(
                        out=S_md[:, :], in0=S_md[:, :],
                        in1=M_t[:, ts(blk, BL)], op=OP.add)
                elif BLK_MID1 <= blk < BLK_HI:
                    nc.gpsimd.tensor_tensor(
                        out=S_hi[:, :], in0=S_hi[:, :],
                        in1=M_t[:, ts(blk, BL)], op=OP.add)
            for jj in sorted(extras):
                for th in extras[jj]:
                    th()

            nc.sync.dma_start(out=out_d[:, :], in_=S_t[:, :])


    _strip_self_waits(nc)
    _rebalance_matmul_waits(nc)
    return nc


def _rebalance_matmul_waits(nc):
    """walrus allows one sync wait per compute instruction.  A matmul that
    ended up with several (e.g. big-mm chunk 0: x-DMA + psum-WAR) gets its
    excess waits moved onto preceding same-engine instructions with a free
    wait slot (their LDWEIGHTS, typically).  Moving a wait earlier in the
    engine stream is always conservative-safe."""
    for fn in nc.m.functions:
        for blk in fn.blocks:
            prev_pe = []
            for inst in blk.instructions:
                tn = type(inst).__name__
                si = inst.sync_info
                if str(inst.engine) not in ("EngineType.PE", "PE"):
                    continue
                waits = list(si.on_wait or []) if si is not None else []
                if tn == "InstMatmult" and len(waits) > 1:
                    import concourse.bass as bass
                    br = bass._bass_rust
                    kept = [waits[0]]
                    for w in waits[1:]:
                        placed = False
                        for p in reversed(prev_pe):
                            psi = p.sync_info
                            if psi is None:
                                p.sync_info = br.SyncInfo(
                                    on_wait=[w], on_update=[])
                                placed = True
                                break
                            if not (psi.on_wait or []):
                                psi.on_wait = [w]
                                placed = True
                                break
                        if not placed:
                            kept.append(w)
                        else:
                            prev_pe.remove(p)
                    si.on_wait = kept
                if tn in ("InstLdweights", "InstNop"):
                    prev_pe.append(inst)
                    prev_pe = prev_pe[-8:]


def _strip_self_waits(nc):
    """Drop redundant same-engine waits (engine streams execute in order) and
    trim tail-drain waits to output-DMA lanes, keeping every compute
    instruction at <=1 sync wait for walrus."""
    import concourse.mybir as mybir

    out_names = set()
    for alloc in nc.m.functions[0].allocations:
        if (
            isinstance(alloc, mybir.MemoryLocationSet)
            and alloc.kind == "ExternalOutput"
        ):
            for ml in alloc.memorylocations:
                out_names.add(ml.name)
    keep_lanes = set()
    for name, inst in nc.inst_map.items():
        if "DMA" not in type(inst).__name__:
            continue
        c = inst.concise()
        if any(f"@{n}" in c.split("in=")[0] for n in out_names):
            for u in (inst.sync_info.on_update or []) if inst.sync_info else []:
                keep_lanes.add(u.ant_name)

    for name, inst in nc.inst_map.items():
        si = inst.sync_info
        if si is None or not si.on_wait or len(si.on_wait) < 2:
            continue
        own = {u.ant_name for u in (si.on_update or [])}
        kept = [w for w in si.on_wait if w.ant_name not in own]
        if "Drain" in type(inst).__name__ and len(kept) > 1:
            kept = [w for w in kept if w.ant_name in keep_lanes]
        if len(kept) != len(si.on_wait):
            si.on_wait = kept


def _prep_shared(W1, b1, Wr, br, W2, b2):
    f32 = np.float32
    W1 = np.asarray(W1, f32); b1 = np.asarray(b1, f32)
    Wr = np.asarray(Wr, f32); br = np.asarray(br, f32)
    W2 = np.asarray(W2, f32); b2 = np.asarray(b2, f32)
    Wrh, Wry = Wr[:, :H], Wr[:, H:]
    Wtil = -0.5 * (Wrh @ W1)                                # [64, 700]
    bt1 = 0.5 * (Wrh @ b1 + br + Wry.sum(axis=1))
    bt2 = 0.5 * (b2 + W2.sum(axis=1))
    # big-mm weights: [chunks, 128(, 2), 64]; feature f = 256c + 128*ko + ki
    Wtp = np.zeros((H, DP), f32)
    Wtp[:, :D] = Wtil
    if USE_DR:
        Wq = np.ascontiguousarray(
            (Wtp * WSCALE).reshape(H, 3, 2, 128).transpose(1, 3, 2, 0)
        ).astype(E4)                                        # [3,128,2,64]
    else:
        Wq = np.ascontiguousarray(
            (Wtp * WSCALE).reshape(H, 6, 128).transpose(1, 2, 0)
        ).astype(E4)                                        # [6,128,64]
    L = np.concatenate([0.5 * Wry.T, 0.5 * W2.T], axis=1).astype(BF16)
    I64 = np.eye(H, dtype=f32).astype(BF16)
    B20 = (1.0 - bt2).reshape(1, C).astype(BF16)
    b64v = (1.0 - bt1).reshape(H, 1).astype(f32)
    return Wq, L, I64, B20, b64v


def _prep_x_core(xbf, k):
    """x slice for core k: steps t in [32k-W, 32k-W+N), zero outside [0,T).

    xbf: (B, T, D) in fp8/bf16 (already cast).  Returns [chunks,128(,2),NCOL].
    """
    t0 = 32 * k - W
    xt = np.zeros((DP, N, BL), xbf.dtype)
    lo = max(0, -t0)
    hi = min(N, T - t0)
    if hi > lo:
        xt[:D, lo:hi] = np.asarray(xbf[:, t0 + lo:t0 + hi, :]).transpose(2, 1, 0)
    xt = xt.reshape(DP, NCOL)
    if USE_DR:
        # feature f = 256c + 128*ko + ki -> [3, 128, 2, NCOL]
        return np.ascontiguousarray(
            xt.reshape(3, 2, 128, NCOL).transpose(0, 2, 1, 3))
    return np.ascontiguousarray(xt.reshape(6, 128, NCOL))


def _ensure_ntff_hook():
    """The RL container's antenv stub lacks axon_hooks; bass_utils imports it
    unconditionally when tracing. Register the ctypes-based hook ourselves."""
    import sys
    import types
    try:
        import antenv
        if "antenv.axon_hooks" in sys.modules:
            return
        mod = types.ModuleType("antenv.axon_hooks")
        _h = [None]
        mod.set_axon_ntff_profile_hook = lambda h: _h.__setitem__(0, h)
        mod.get_axon_ntff_profile_hook = lambda: _h[0]
        sys.modules["antenv.axon_hooks"] = mod
        antenv.axon_hooks = mod
        try:
            from trn_agent_boot.trn_boot import _ntff_profile_via_ctypes
            mod.set_axon_ntff_profile_hook(
                _ntff_profile_via_ctypes("/opt/axon/libaxon_pjrt.so")
            )
        except Exception:
            pass
    except Exception:
        pass


def _combine(res_list):
    """Host combine: sum valid buckets per core -> m2 counts -> output."""
    count = np.zeros((C, BL), np.float64)
    s1count = np.zeros((H, BL), np.float64)
    for k, r in enumerate(res_list):
        S = np.asarray(r["out"], np.float64)        # [84, 3*256]
        lo, md, hi = S[:, 0:BL], S[:, BL:2 * BL], S[:, 2 * BL:3 * BL]
        if k == 0:
            v = md + hi
        elif k == NCORES - 1:
            v = lo + md
        else:
            v = lo + md + hi
        count += v[H:P]
        s1count += v[0:H]
    out = (235.0 - count) / 235.0                   # mean s2, (20, 256)
    return out.astype(np.float32), s1count


def kernel(x, W1, b1, Wr, br, W2, b2):
    from concourse.bass_utils import run_bass_kernel_spmd

    _ensure_ntff_hook()

    if "nc" not in _CACHE:
        _CACHE["nc"] = _build_nc()
    nc = _CACHE["nc"]

    Wq, L, I64, B20, b64v = _prep_shared(W1, b1, Wr, br, W2, b2)

    x = np.asarray(x, np.float32)
    xcast = x.astype(E4)
    in_maps = []
    for k in range(NCORES):
        in_maps.append({
            "xq": _prep_x_core(xcast, k),
            "Wq": Wq, "Lw": L, "I64d": I64, "B20d": B20, "b64": b64v,
        })

    res = run_bass_kernel_spmd(nc, in_maps, core_ids=list(range(NCORES)))
    _CACHE["last_results"] = res
    out, s1count = _combine([r for r in res.results])
    _CACHE["s1count"] = s1count
    return np.ascontiguousarray(out.T)              # (256, 20)


# revision 19
# speedup vs baseline: 1.2575x; 1.2575x over previous
"""Trainium2 Bass kernel for nn_CompNet (spiking LIF RNN) — V2.

Strategy vs. the V1 baseline (191 us):
  * Time-axis speculative parallelism: T=250 steps split into 8 slices of 32
    (T padded to 256); each core runs its slice plus a W=12-step warmup from
    zero state.  LIF state decays 2^-1 per step and hard-resets, so the
    trajectory reconverges exactly within 12 steps (validated in numpy:
    0 spike flips at W=16, <10 harmless flips at W=12).  Serial-loop length
    drops 250 -> 44 per core; every core carries the full batch B=256.
  * Resident PE weights via tile_position (no per-step LDWEIGHTS):
      rows 0-63   cols 0-83 : L      (recurrent + readout, loaded once)
      rows 64-127 cols 0-63 : I64    (E-injection identity, loaded once)
      row  64     cols 64-83: B20    (LIF2 constant row, reloaded after
                                      big-mm bursts that clobber cols 64-127)
      rows 0-127  cols 64-127: big-mm weights (fp8 DoubleRow, rotating)
    Loop matmuls carry ins.ldweights=False so the stationary operand stays.
  * Per step j: PE  ps = E_j-inject + B20-inject + L@M_j   (3 MMs, only the
    L@M one waits on the mask); DVE  mask/u/cu (bf16 state); GPSIMD
    accumulates LIF2 (and LIF1, as a self-check channel) mask counts into
    three j-range buckets so the host can drop warmup/out-of-range steps
    per core without breaking SPMD uniformity.
  * Feedforward drive E = Wtil@x (+bias) in fp8 DoubleRow (wtil scaled by 64
    host-side, un-scaled in the ACT evacuation), interleaved with the loop.
    fp8 x halves DMA to ~8.7 MB/core; numpy-validated: v2 margin to
    threshold stays ~0.45, output unchanged.

Math (same negated encoding as V1):
  qsum_j = (1-bt) - 0.5*Wrh@W1@x_j - 0.5*[Wry;W2]@m_{j-1};  m = (v < 1)
  spike test: v_j < 1  <=>  0.5*cu_{j-1} < qsum_j
  u = 0.5*cu - qsum = v - 1;  cu' = (u+1)*m
Stats: sum of m2 (rows 64-83) over the valid window; out = (235 - S)/235.
"""

import numpy as np
import ml_dtypes

BF16 = ml_dtypes.bfloat16
E4 = ml_dtypes.float8_e4m3

B, T, D, H, C = 256, 250, 700, 64, 20
NCORES = 8
S_SPLIT = 8               # time slices
SEG = 32                  # steps per slice (T padded to 256)
W = 10                    # warmup steps
N = SEG + W               # 44 real steps per core
NITER = N + 1             # +1 loop iter for the lagged LIF2 mask tail
BL = B                    # 256 batch columns per step block
NCOL = N * BL             # 11264 drive columns per core
P = H + C                 # 84 stacked rows (LIF1 + LIF2)
DP = 768                  # padded feature dim (3 fp8-DR chunks of 256)
WSCALE = 64.0             # fp8 weight pre-scale (undone in ACT evacuation)
VTH_INIT = 2.0e9          # suppresses the phantom LIF2 step at j=0

# stats buckets (block index = local step + 2)
BLK_LO, BLK_MID0, BLK_MID1, BLK_HI = W + 2, W + 17, W + 28, N + 2

USE_DR = False            # fp8-normal measured == DR throughput; simpler
USE_RESIDENT = True       # ins.ldweights=False resident-weight loop MMs

_CACHE = {}


def _build_nc():
    import concourse.bass as bass
    import concourse.mybir as mybir
    from concourse.tile import TileContext

    dt = mybir.dt
    AF = mybir.ActivationFunctionType
    OP = mybir.AluOpType
    PM = mybir.MatmulPerfMode
    ts = bass.ts

    nc = bass.Bass(
        "TRN2", target_bir_lowering=False, debug=False,
        detect_race_conditions=False,
    )

    xdt = dt.float8e4
    if USE_DR:
        xq = nc.dram_tensor("xq", [3, 128, 2, NCOL], xdt, kind="ExternalInput").ap()
        Wq = nc.dram_tensor("Wq", [3, 128, 2, H], xdt, kind="ExternalInput").ap()
    else:
        xq = nc.dram_tensor("xq", [128, 6, NCOL], xdt, kind="ExternalInput").ap()
        Wq = nc.dram_tensor("Wq", [128, 6, H], xdt, kind="ExternalInput").ap()
    Lw = nc.dram_tensor("Lw", [H, P], dt.bfloat16, kind="ExternalInput").ap()
    I64d = nc.dram_tensor("I64d", [H, H], dt.bfloat16, kind="ExternalInput").ap()
    B20d = nc.dram_tensor("B20d", [1, C], dt.bfloat16, kind="ExternalInput").ap()
    b64 = nc.dram_tensor("b64", [H, 1], dt.float32, kind="ExternalInput").ap()
    out_d = nc.dram_tensor("out", [P, 3 * BL], dt.float32, kind="ExternalOutput").ap()

    res_mms = []

    # x DMA pieces: 1024-col granularity, emitted progressively
    XP = 1024
    NXP = NCOL // XP          # 11 pieces per chunk
    NPIECE = NCOL // 512      # 22 big-mm pieces

    with TileContext(nc) as tc:
        with (
            tc.tile_pool(name="const", bufs=1) as cp,
            tc.tile_pool(name="psL", bufs=3, space="PSUM") as psL,
            tc.tile_pool(name="psF", bufs=2, space="PSUM") as psF,
            tc.tile_pool(name="wk", bufs=3) as wp,
        ):
            # ---- persistent tiles ----
            L_t = cp.tile([H, P], dt.bfloat16, tag="L")
            IF_t = cp.tile([128, H], dt.bfloat16, tag="I")
            I_t = IF_t[64:128, :]
            BF_t = cp.tile([128, C], dt.bfloat16, tag="B20")
            B20_t = BF_t[64:65, :]
            ON_t = cp.tile([128, BL], dt.bfloat16, tag="ones")
            ones_t = ON_t[64:65, :]
            EF_t = cp.tile([128, NCOL], dt.bfloat16, tag="EF")
            E_t = EF_t[64:128, :]
            M_t = cp.tile([P, (NITER + 1) * BL], dt.bfloat16, tag="M")
            cu0 = cp.tile([P, BL], dt.float32, tag="cu0")
            bb_t = cp.tile([128, 1], dt.float32, tag="bb")
            b64_t = bb_t[64:128, :]
            S_t = cp.tile([P, 3 * BL], dt.float32, tag="St")
            S_lo = S_t[:, 0:BL]
            S_md = S_t[:, BL:2 * BL]
            S_hi = S_t[:, 2 * BL:3 * BL]
            if USE_DR:
                xts = [cp.tile([128, 2, NCOL], xdt, tag=f"x{c}", name=f"xts{c}")
                       for c in range(3)]
                wts = [cp.tile([128, 2, H], xdt, tag=f"w{c}", name=f"wts{c}")
                       for c in range(3)]
            else:
                xts_t = cp.tile([128, 6, NCOL], xdt, tag="xts")
                wts_t = cp.tile([128, 6, H], xdt, tag="wts")

            # ---- prologue DMAs + inits ----
            def emit_xdma_cols(c0, w):
                nc.sync.dma_start(out=xts_t[:, :, c0:c0 + w],
                                  in_=xq[:, :, c0:c0 + w])

            emit_xdma_cols(0, 512)
            nc.sync.dma_start(out=wts_t[:, :, :], in_=Wq[:, :, :])
            nc.sync.dma_start(out=L_t[:, :], in_=Lw[:, :])
            nc.sync.dma_start(out=I_t[:, :], in_=I64d[:, :])
            nc.sync.dma_start(out=BF_t[64:65, :], in_=B20d[:, :])
            nc.sync.dma_start(out=bb_t[64:128, :], in_=b64[:, :])

            # absorb the b64 DMA wait on ACT here so the per-piece evacuation
            # activations carry only their PE wait (walrus 1-wait limit)
            btc = cp.tile([128, 1], dt.float32, tag="btc")
            nc.scalar.activation(
                out=btc[64:128, :], in_=b64_t[:, :],
                func=AF.Copy, bias=0.0, scale=1.0)

            nc.vector.memset(ON_t[64:65, :], 1.0)
            nc.vector.memset(M_t[0:H, 0:BL], 1.0)      # m_{-1}=1 (y=0)
            nc.vector.memset(M_t[H:P, 0:BL], 0.0)
            nc.vector.memset(cu0[0:H, :], 0.0)
            nc.vector.memset(cu0[H:P, :], VTH_INIT)
            nc.gpsimd.memset(S_t[:, :], 0.0)

            def emit_piece(p):
                """Big-mm piece: 512 drive columns -> E (PSUM->ACT->SBUF)."""
                c0 = p * 512
                pf = psF.tile([128, 512], dt.float32, tag="pf")
                for c in range(6):
                    nc.tensor.matmul(
                        out=pf[64:128, :], lhsT=wts_t[:, c, :],
                        rhs=xts_t[:, c, c0:c0 + 512],
                        start=(c == 0), stop=(c == 5),
                        tile_position=(0, 64))
                nc.scalar.activation(
                    out=EF_t[64:128, c0:c0 + 512], in_=pf[64:128, :],
                    func=AF.Identity, bias=b64_t[:, 0:1], scale=1.0 / WSCALE)

            # prologue: first x pieces + big-mm pieces + resident weights
            emit_xdma_cols(512, 1536)
            nc.tensor.ldweights(L_t[:, :], tile_position=(0, 0))
            nc.tensor.ldweights(IF_t[64:128, :], tile_position=(64, 0))
            emit_piece(0)
            emit_piece(1)

            extras = {}
            for p in range(2, NPIECE):
                extras.setdefault(max(0, 2 * p - 4), []).append(
                    lambda p=p: emit_piece(p))
            c0 = 2048
            while c0 < NCOL:
                w = min(1024, NCOL - c0)
                it = max(0, 2 * (c0 // 512) - 10)
                extras.setdefault(it, []).append(
                    lambda c0=c0, w=w: emit_xdma_cols(c0, w))
                c0 += w

            # ---- the serial LIF loop ----
            cu_prev = cu0
            for j in range(NITER):
                for th in extras.pop(j, []):
                    th()
                ps = psL.tile([P, BL], dt.float32, tag="ps")
                ej = j if j < N else 0
                # wait discipline (walrus: one sync wait per compute inst):
                #   i2 self-loads B20 (waits: psum-bank WAR on DVE)
                #   i1 resident I64   (waits: ACT wrote E block)
                #   i3 resident L     (waits: DVE mask block j)
                # start=True on BOTH injects: has_written/pending-zero is
                # per-partition (each start covers its own out partitions),
                # i3 then accumulates across all 84.
                i2 = nc.tensor.matmul(
                    out=ps[H:P, :], lhsT=B20_t[:, :], rhs=ones_t[:, :],
                    start=True, stop=False, tile_position=(64, 64),
                    skip_group_check=True)
                i1 = nc.tensor.matmul(
                    out=ps[0:H, :], lhsT=I_t[:, :], rhs=E_t[:, ts(ej, BL)],
                    start=True, stop=False, tile_position=(64, 0),
                    skip_group_check=True)
                i3 = nc.tensor.matmul(
                    out=ps[:, :], lhsT=L_t[:, :], rhs=M_t[0:H, ts(j, BL)],
                    start=False, stop=True, tile_position=(0, 0),
                    skip_group_check=True)
                res_mms.extend([i1, i3])
                nc.vector.scalar_tensor_tensor(
                    out=M_t[:, ts(j + 1, BL)], in0=cu_prev[:, :], scalar=0.5,
                    in1=ps[:, :], op0=OP.mult, op1=OP.is_lt)
                if j < NITER - 1:
                    u = wp.tile([P, BL], dt.float32, tag="u")
                    nc.vector.scalar_tensor_tensor(
                        out=u[:, :], in0=cu_prev[:, :], scalar=0.5,
                        in1=ps[:, :], op0=OP.mult, op1=OP.subtract)
                    cu = wp.tile([P, BL], dt.float32, tag="cu")
                    nc.vector.scalar_tensor_tensor(
                        out=cu[:, :], in0=u[:, :], scalar=1.0,
                        in1=M_t[:, ts(j + 1, BL)], op0=OP.add, op1=OP.mult)
                    cu_prev = cu
                # stats: mask block j+1 into its j-range bucket (full 84 rows;
                # rows 0-63 double as a host-side self-check channel)
                blk = j + 1
                if BLK_LO <= blk < BLK_MID0:
                    nc.gpsimd.tensor_tensor(
                        out=S_lo[:, :], in0=S_lo[:, :],
                        in1=M_t[:, ts(blk, BL)], op=OP.add)
                elif BLK_MID0 <= blk < BLK_MID1:
                    nc.gpsimd.tensor_tensor(
                        out=S_md[:, :], in0=S_md[:, :],
                        in1=M_t[:, ts(blk, BL)], op=OP.add)
                elif BLK_MID1 <= blk < BLK_HI:
                    nc.gpsimd.tensor_tensor(
                        out=S_hi[:, :], in0=S_hi[:, :],
                        in1=M_t[:, ts(blk, BL)], op=OP.add)
            for jj in sorted(extras):
                for th in extras[jj]:
                    th()

            nc.sync.dma_start(out=out_d[:, :], in_=S_t[:, :])


    _strip_self_waits(nc)
    _rebalance_matmul_waits(nc)
    return nc


def _rebalance_matmul_waits(nc):
    """walrus allows one sync wait per compute instruction.  A matmul that
    ended up with several (e.g. big-mm chunk 0: x-DMA + psum-WAR) gets its
    excess waits moved onto preceding same-engine instructions with a free
    wait slot (their LDWEIGHTS, typically).  Moving a wait earlier in the
    engine stream is always conservative-safe."""
    for fn in nc.m.functions:
        for blk in fn.blocks:
            prev_pe = []
            for inst in blk.instructions:
                tn = type(inst).__name__
                si = inst.sync_info
                if str(inst.engine) not in ("EngineType.PE", "PE"):
                    continue
                waits = list(si.on_wait or []) if si is not None else []
                if tn == "InstMatmult" and len(waits) > 1:
                    import concourse.bass as bass
                    br = bass._bass_rust
                    kept = [waits[0]]
                    for w in waits[1:]:
                        placed = False
                        for p in reversed(prev_pe):
                            psi = p.sync_info
                            if psi is None:
                                p.sync_info = br.SyncInfo(
                                    on_wait=[w], on_update=[])
                                placed = True
                                break
                            if not (psi.on_wait or []):
                                psi.on_wait = [w]
                                placed = True
                                break
                        if not placed:
                            kept.append(w)
                        else:
                            prev_pe.remove(p)
                    si.on_wait = kept
                if tn in ("InstLdweights", "InstNop"):
                    prev_pe.append(inst)
                    prev_pe = prev_pe[-8:]


def _strip_self_waits(nc):
    """Drop redundant same-engine waits (engine streams execute in order) and
    trim tail-drain waits to output-DMA lanes, keeping every compute
    instruction at <=1 sync wait for walrus."""
    import concourse.mybir as mybir

    out_names = set()
    for alloc in nc.m.functions[0].allocations:
        if (
            isinstance(alloc, mybir.MemoryLocationSet)
            and alloc.kind == "ExternalOutput"
        ):
            for ml in alloc.memorylocations:
                out_names.add(ml.name)
    keep_lanes = set()
    for name, inst in nc.inst_map.items():
        if "DMA" not in type(inst).__name__:
            continue
        c = inst.concise()
        if any(f"@{n}" in c.split("in=")[0] for n in out_names):
            for u in (inst.sync_info.on_update or []) if inst.sync_info else []:
                keep_lanes.add(u.ant_name)

    for name, inst in nc.inst_map.items():
        si = inst.sync_info
        if si is None or not si.on_wait or len(si.on_wait) < 2:
            continue
        own = {u.ant_name for u in (si.on_update or [])}
        kept = [w for w in si.on_wait if w.ant_name not in own]
        if "Drain" in type(inst).__name__ and len(kept) > 1:
            kept = [w for w in kept if w.ant_name in keep_lanes]
        if len(kept) != len(si.on_wait):
            si.on_wait = kept


def _prep_shared(W1, b1, Wr, br, W2, b2):
    f32 = np.float32
    W1 = np.asarray(W1, f32); b1 = np.asarray(b1, f32)
    Wr = np.asarray(Wr, f32); br = np.asarray(br, f32)
    W2 = np.asarray(W2, f32); b2 = np.asarray(b2, f32)
    Wrh, Wry = Wr[:, :H], Wr[:, H:]
    Wtil = -0.5 * (Wrh @ W1)                                # [64, 700]
    bt1 = 0.5 * (Wrh @ b1 + br + Wry.sum(axis=1))
    bt2 = 0.5 * (b2 + W2.sum(axis=1))
    # big-mm weights: [chunks, 128(, 2), 64]; feature f = 256c + 128*ko + ki
    Wtp = np.zeros((H, DP), f32)
    Wtp[:, :D] = Wtil
    if USE_DR:
        Wq = np.ascontiguousarray(
            (Wtp * WSCALE).reshape(H, 3, 2, 128).transpose(1, 3, 2, 0)
        ).astype(E4)                                        # [3,128,2,64]
    else:
        Wq = np.ascontiguousarray(
            (Wtp * WSCALE).reshape(H, 6, 128).transpose(2, 1, 0)
        ).astype(E4)                                        # [128,6,64]
    L = np.concatenate([0.5 * Wry.T, 0.5 * W2.T], axis=1).astype(BF16)
    I64 = np.eye(H, dtype=f32).astype(BF16)
    B20 = (1.0 - bt2).reshape(1, C).astype(BF16)
    b64v = (1.0 - bt1).reshape(H, 1).astype(f32)
    return Wq, L, I64, B20, b64v


def _prep_x_core(xbf, k):
    """x slice for core k: steps t in [32k-W, 32k-W+N), zero outside [0,T).

    xbf: (B, T, D) in fp8/bf16 (already cast).  Returns [chunks,128(,2),NCOL].
    """
    t0 = 32 * k - W
    xt = np.zeros((DP, N, BL), xbf.dtype)
    lo = max(0, -t0)
    hi = min(N, T - t0)
    if hi > lo:
        xt[:D, lo:hi] = np.asarray(xbf[:, t0 + lo:t0 + hi, :]).transpose(2, 1, 0)
    xt = xt.reshape(DP, NCOL)
    if USE_DR:
        # feature f = 256c + 128*ko + ki -> [3, 128, 2, NCOL]
        return np.ascontiguousarray(
            xt.reshape(3, 2, 128, NCOL).transpose(0, 2, 1, 3))
    return np.ascontiguousarray(xt.reshape(6, 128, NCOL).transpose(1, 0, 2))


def _ensure_ntff_hook():
    """The RL container's antenv stub lacks axon_hooks; bass_utils imports it
    unconditionally when tracing. Register the ctypes-based hook ourselves."""
    import sys
    import types
    try:
        import antenv
        if "antenv.axon_hooks" in sys.modules:
            return
        mod = types.ModuleType("antenv.axon_hooks")
        _h = [None]
        mod.set_axon_ntff_profile_hook = lambda h: _h.__setitem__(0, h)
        mod.get_axon_ntff_profile_hook = lambda: _h[0]
        sys.modules["antenv.axon_hooks"] = mod
        antenv.axon_hooks = mod
        try:
            from trn_agent_boot.trn_boot import _ntff_profile_via_ctypes
            mod.set_axon_ntff_profile_hook(
                _ntff_profile_via_ctypes("/opt/axon/libaxon_pjrt.so")
            )
        except Exception:
            pass
    except Exception:
        pass


def _combine(res_list):
    """Host combine: sum valid buckets per core -> m2 counts -> output."""
    count = np.zeros((C, BL), np.float64)
    s1count = np.zeros((H, BL), np.float64)
    for k, r in enumerate(res_list):
        S = np.asarray(r["out"], np.float64)        # [84, 3*256]
        lo, md, hi = S[:, 0:BL], S[:, BL:2 * BL], S[:, 2 * BL:3 * BL]
        if k == 0:
            v = md + hi
        elif k == NCORES - 1:
            v = lo + md
        else:
            v = lo + md + hi
        count += v[H:P]
        s1count += v[0:H]
    out = (235.0 - count) / 235.0                   # mean s2, (20, 256)
    return out.astype(np.float32), s1count


def kernel(x, W1, b1, Wr, br, W2, b2):
    from concourse.bass_utils import run_bass_kernel_spmd

    _ensure_ntff_hook()

    if "nc" not in _CACHE:
        _CACHE["nc"] = _build_nc()
    nc = _CACHE["nc"]

    Wq, L, I64, B20, b64v = _prep_shared(W1, b1, Wr, br, W2, b2)

    x = np.asarray(x, np.float32)
    xcast = x.astype(E4)
    in_maps = []
    for k in range(NCORES):
        in_maps.append({
            "xq": _prep_x_core(xcast, k),
            "Wq": Wq, "Lw": L, "I64d": I64, "B20d": B20, "b64": b64v,
        })

    res = run_bass_kernel_spmd(nc, in_maps, core_ids=list(range(NCORES)))
    _CACHE["last_results"] = res
    out, s1count = _combine([r for r in res.results])
    _CACHE["s1count"] = s1count
    return np.ascontiguousarray(out.T)              # (256, 20)


# revision 20
# speedup vs baseline: 1.3227x; 1.0518x over previous
"""Trainium2 Bass kernel for nn_CompNet (spiking LIF RNN) — V2.

Strategy vs. the V1 baseline (191 us):
  * Time-axis speculative parallelism: T=250 steps split into 8 slices of 32
    (T padded to 256); each core runs its slice plus a W=12-step warmup from
    zero state.  LIF state decays 2^-1 per step and hard-resets, so the
    trajectory reconverges exactly within 12 steps (validated in numpy:
    0 spike flips at W=16, <10 harmless flips at W=12).  Serial-loop length
    drops 250 -> 44 per core; every core carries the full batch B=256.
  * Resident PE weights via tile_position (no per-step LDWEIGHTS):
      rows 0-63   cols 0-83 : L      (recurrent + readout, loaded once)
      rows 64-127 cols 0-63 : I64    (E-injection identity, loaded once)
      row  64     cols 64-83: B20    (LIF2 constant row, reloaded after
                                      big-mm bursts that clobber cols 64-127)
      rows 0-127  cols 64-127: big-mm weights (fp8 DoubleRow, rotating)
    Loop matmuls carry ins.ldweights=False so the stationary operand stays.
  * Per step j: PE  ps = E_j-inject + B20-inject + L@M_j   (3 MMs, only the
    L@M one waits on the mask); DVE  mask/u/cu (bf16 state); GPSIMD
    accumulates LIF2 (and LIF1, as a self-check channel) mask counts into
    three j-range buckets so the host can drop warmup/out-of-range steps
    per core without breaking SPMD uniformity.
  * Feedforward drive E = Wtil@x (+bias) in fp8 DoubleRow (wtil scaled by 64
    host-side, un-scaled in the ACT evacuation), interleaved with the loop.
    fp8 x halves DMA to ~8.7 MB/core; numpy-validated: v2 margin to
    threshold stays ~0.45, output unchanged.

Math (same negated encoding as V1):
  qsum_j = (1-bt) - 0.5*Wrh@W1@x_j - 0.5*[Wry;W2]@m_{j-1};  m = (v < 1)
  spike test: v_j < 1  <=>  0.5*cu_{j-1} < qsum_j
  u = 0.5*cu - qsum = v - 1;  cu' = (u+1)*m
Stats: sum of m2 (rows 64-83) over the valid window; out = (235 - S)/235.
"""

import numpy as np
import ml_dtypes

BF16 = ml_dtypes.bfloat16
E4 = ml_dtypes.float8_e4m3

B, T, D, H, C = 256, 250, 700, 64, 20
NCORES = 8
S_SPLIT = 8               # time slices
SEG = 32                  # steps per slice (T padded to 256)
W = 10                    # warmup steps
N = SEG + W               # 44 real steps per core
NITER = N + 1             # +1 loop iter for the lagged LIF2 mask tail
BL = B                    # 256 batch columns per step block
NCOL = N * BL             # 11264 drive columns per core
P = H + C                 # 84 stacked rows (LIF1 + LIF2)
DP = 768                  # padded feature dim (3 fp8-DR chunks of 256)
WSCALE = 64.0             # fp8 weight pre-scale (undone in ACT evacuation)
VTH_INIT = 2.0e9          # suppresses the phantom LIF2 step at j=0

# stats buckets (block index = local step + 2)
BLK_LO, BLK_MID0, BLK_MID1, BLK_HI = W + 2, W + 17, W + 28, N + 2

USE_DR = False            # fp8-normal measured == DR throughput; simpler
USE_RESIDENT = True       # ins.ldweights=False resident-weight loop MMs

_CACHE = {}


def _build_nc():
    import concourse.bass as bass
    import concourse.mybir as mybir
    from concourse.tile import TileContext

    dt = mybir.dt
    AF = mybir.ActivationFunctionType
    OP = mybir.AluOpType
    PM = mybir.MatmulPerfMode
    ts = bass.ts

    nc = bass.Bass(
        "TRN2", target_bir_lowering=False, debug=False,
        detect_race_conditions=False,
    )

    xdt = dt.float8e4
    if USE_DR:
        xq = nc.dram_tensor("xq", [3, 128, 2, NCOL], xdt, kind="ExternalInput").ap()
        Wq = nc.dram_tensor("Wq", [3, 128, 2, H], xdt, kind="ExternalInput").ap()
    else:
        xq = nc.dram_tensor("xq", [128, 6, NCOL], xdt, kind="ExternalInput").ap()
        Wq = nc.dram_tensor("Wq", [128, 6, H], xdt, kind="ExternalInput").ap()
    Lw = nc.dram_tensor("Lw", [H, P], dt.bfloat16, kind="ExternalInput").ap()
    I64d = nc.dram_tensor("I64d", [H, H], dt.bfloat16, kind="ExternalInput").ap()
    B20d = nc.dram_tensor("B20d", [1, C], dt.bfloat16, kind="ExternalInput").ap()
    b64 = nc.dram_tensor("b64", [H, 1], dt.float32, kind="ExternalInput").ap()
    out_d = nc.dram_tensor("out", [P, 3 * BL], dt.float32, kind="ExternalOutput").ap()

    res_mms = []

    # x DMA pieces: 1024-col granularity, emitted progressively
    XP = 1024
    NXP = NCOL // XP          # 11 pieces per chunk
    NPIECE = NCOL // 512      # 22 big-mm pieces

    with TileContext(nc) as tc:
        with (
            tc.tile_pool(name="const", bufs=1) as cp,
            tc.tile_pool(name="psL", bufs=3, space="PSUM") as psL,
            tc.tile_pool(name="psF", bufs=2, space="PSUM") as psF,
            tc.tile_pool(name="wk", bufs=3) as wp,
        ):
            # ---- persistent tiles ----
            L_t = cp.tile([H, P], dt.bfloat16, tag="L")
            IF_t = cp.tile([128, H], dt.bfloat16, tag="I")
            I_t = IF_t[64:128, :]
            BF_t = cp.tile([128, C], dt.bfloat16, tag="B20")
            B20_t = BF_t[64:65, :]
            ON_t = cp.tile([128, 2 * BL], dt.bfloat16, tag="ones")
            ones_t = ON_t[64:65, :]
            EF_t = cp.tile([128, NCOL], dt.bfloat16, tag="EF")
            E_t = EF_t[64:128, :]
            M_t = cp.tile([P, (NITER + 1) * BL], dt.bfloat16, tag="M")
            cu0 = cp.tile([P, BL], dt.float32, tag="cu0")
            bb_t = cp.tile([128, 1], dt.float32, tag="bb")
            b64_t = bb_t[64:128, :]
            S_t = cp.tile([P, 3 * BL], dt.float32, tag="St")
            S_lo = S_t[:, 0:BL]
            S_md = S_t[:, BL:2 * BL]
            S_hi = S_t[:, 2 * BL:3 * BL]
            if USE_DR:
                xts = [cp.tile([128, 2, NCOL], xdt, tag=f"x{c}", name=f"xts{c}")
                       for c in range(3)]
                wts = [cp.tile([128, 2, H], xdt, tag=f"w{c}", name=f"wts{c}")
                       for c in range(3)]
            else:
                xts_t = cp.tile([128, 6, NCOL], xdt, tag="xts")
                wts_t = cp.tile([128, 6, H], xdt, tag="wts")

            # ---- prologue DMAs + inits ----
            def emit_xdma_cols(c0, w):
                nc.sync.dma_start(out=xts_t[:, :, c0:c0 + w],
                                  in_=xq[:, :, c0:c0 + w])

            emit_xdma_cols(0, 512)
            nc.sync.dma_start(out=wts_t[:, :, :], in_=Wq[:, :, :])
            nc.sync.dma_start(out=L_t[:, :], in_=Lw[:, :])
            nc.sync.dma_start(out=I_t[:, :], in_=I64d[:, :])
            nc.sync.dma_start(out=BF_t[64:65, :], in_=B20d[:, :])
            nc.sync.dma_start(out=bb_t[64:128, :], in_=b64[:, :])

            # absorb the b64 DMA wait on ACT here so the per-piece evacuation
            # activations carry only their PE wait (walrus 1-wait limit)
            btc = cp.tile([128, 1], dt.float32, tag="btc")
            nc.scalar.activation(
                out=btc[64:128, :], in_=b64_t[:, :],
                func=AF.Copy, bias=0.0, scale=1.0)

            nc.vector.memset(ON_t[64:65, :], 1.0)
            nc.vector.memset(M_t[0:H, 0:BL], 1.0)      # m_{-1}=1 (y=0)
            nc.vector.memset(M_t[H:P, 0:BL], 0.0)
            nc.vector.memset(cu0[0:H, :], 0.0)
            nc.vector.memset(cu0[H:P, :], VTH_INIT)
            nc.gpsimd.memset(S_t[:, :], 0.0)

            def emit_piece(p):
                """Big-mm piece: 512 drive columns -> E (PSUM->ACT->SBUF)."""
                c0 = p * 512
                pf = psF.tile([128, 512], dt.float32, tag="pf")
                for c in range(6):
                    nc.tensor.matmul(
                        out=pf[64:128, :], lhsT=wts_t[:, c, :],
                        rhs=xts_t[:, c, c0:c0 + 512],
                        start=(c == 0), stop=(c == 5),
                        tile_position=(0, 64))
                nc.scalar.activation(
                    out=EF_t[64:128, c0:c0 + 512], in_=pf[64:128, :],
                    func=AF.Identity, bias=b64_t[:, 0:1], scale=1.0 / WSCALE)

            # prologue: first x pieces + big-mm pieces + resident weights
            emit_xdma_cols(512, 1536)
            nc.tensor.ldweights(L_t[:, :], tile_position=(0, 0))
            nc.tensor.ldweights(IF_t[64:128, :], tile_position=(64, 0))
            emit_piece(0)
            emit_piece(1)

            extras = {}
            for p in range(2, NPIECE):
                extras.setdefault(max(0, 2 * p - 4), []).append(
                    lambda p=p: emit_piece(p))
            c0 = 2048
            while c0 < NCOL:
                w = min(1024, NCOL - c0)
                it = max(0, 2 * (c0 // 512) - 10)
                extras.setdefault(it, []).append(
                    lambda c0=c0, w=w: emit_xdma_cols(c0, w))
                c0 += w

            # ---- the serial LIF loop (paired psum banks: 2 steps/bank) ----
            cu_prev = cu0
            ps_pair = None
            for j in range(NITER):
                for th in extras.pop(j, []):
                    th()
                if j % 2 == 0:
                    wq = 2 * BL if j + 1 < NITER else BL
                    ej0 = j if j < N else 0
                    ps_pair = psL.tile([P, 2 * BL], dt.float32, tag="ps")
                    # i2p self-loads B20 (waits: psum-bank WAR on DVE)
                    # i1p resident I64  (waits: ACT wrote E blocks)
                    # per-step i3       (waits: DVE mask block j)
                    nc.tensor.matmul(
                        out=ps_pair[H:P, 0:wq], lhsT=B20_t[:, :],
                        rhs=ones_t[:, 0:wq], start=True, stop=False,
                        tile_position=(64, 64), skip_group_check=True)
                    nc.tensor.matmul(
                        out=ps_pair[0:H, 0:wq], lhsT=I_t[:, :],
                        rhs=E_t[:, ej0 * BL:ej0 * BL + wq], start=True,
                        stop=False, tile_position=(64, 0),
                        skip_group_check=True)
                ps = ps_pair[:, (j % 2) * BL:(j % 2) * BL + BL]
                nc.tensor.matmul(
                    out=ps[:, :], lhsT=L_t[:, :], rhs=M_t[0:H, ts(j, BL)],
                    start=False, stop=(j % 2 == 1 or j == NITER - 1),
                    tile_position=(0, 0), skip_group_check=True)
                nc.vector.scalar_tensor_tensor(
                    out=M_t[:, ts(j + 1, BL)], in0=cu_prev[:, :], scalar=0.5,
                    in1=ps[:, :], op0=OP.mult, op1=OP.is_lt)
                if j < NITER - 1:
                    u = wp.tile([P, BL], dt.float32, tag="u")
                    nc.vector.scalar_tensor_tensor(
                        out=u[:, :], in0=cu_prev[:, :], scalar=0.5,
                        in1=ps[:, :], op0=OP.mult, op1=OP.subtract)
                    cu = wp.tile([P, BL], dt.float32, tag="cu")
                    nc.vector.scalar_tensor_tensor(
                        out=cu[:, :], in0=u[:, :], scalar=1.0,
                        in1=M_t[:, ts(j + 1, BL)], op0=OP.add, op1=OP.mult)
                    cu_prev = cu
                # stats: mask block j+1 into its j-range bucket (full 84 rows;
                # rows 0-63 double as a host-side self-check channel)
                blk = j + 1
                if BLK_LO <= blk < BLK_MID0:
                    nc.gpsimd.tensor_tensor(
                        out=S_lo[:, :], in0=S_lo[:, :],
                        in1=M_t[:, ts(blk, BL)], op=OP.add)
                elif BLK_MID0 <= blk < BLK_MID1:
                    nc.gpsimd.tensor_tensor(
                        out=S_md[:, :], in0=S_md[:, :],
                        in1=M_t[:, ts(blk, BL)], op=OP.add)
                elif BLK_MID1 <= blk < BLK_HI:
                    nc.gpsimd.tensor_tensor(
                        out=S_hi[:, :], in0=S_hi[:, :],
                        in1=M_t[:, ts(blk, BL)], op=OP.add)
            for jj in sorted(extras):
                for th in extras[jj]:
                    th()

            nc.sync.dma_start(out=out_d[:, :], in_=S_t[:, :])


    _strip_self_waits(nc)
    _rebalance_matmul_waits(nc)
    return nc


def _rebalance_matmul_waits(nc):
    """walrus allows one sync wait per compute instruction.  A matmul that
    ended up with several (e.g. big-mm chunk 0: x-DMA + psum-WAR) gets its
    excess waits moved onto preceding same-engine instructions with a free
    wait slot (their LDWEIGHTS, typically).  Moving a wait earlier in the
    engine stream is always conservative-safe."""
    for fn in nc.m.functions:
        for blk in fn.blocks:
            prev_pe = []
            for inst in blk.instructions:
                tn = type(inst).__name__
                si = inst.sync_info
                if str(inst.engine) not in ("EngineType.PE", "PE"):
                    continue
                waits = list(si.on_wait or []) if si is not None else []
                if tn == "InstMatmult" and len(waits) > 1:
                    import concourse.bass as bass
                    br = bass._bass_rust
                    kept = [waits[0]]
                    for w in waits[1:]:
                        placed = False
                        for p in reversed(prev_pe):
                            psi = p.sync_info
                            if psi is None:
                                p.sync_info = br.SyncInfo(
                                    on_wait=[w], on_update=[])
                                placed = True
                                break
                            if not (psi.on_wait or []):
                                psi.on_wait = [w]
                                placed = True
                                break
                        if not placed:
                            kept.append(w)
                        else:
                            prev_pe.remove(p)
                    si.on_wait = kept
                if tn in ("InstLdweights", "InstNop"):
                    prev_pe.append(inst)
                    prev_pe = prev_pe[-8:]


def _strip_self_waits(nc):
    """Drop redundant same-engine waits (engine streams execute in order) and
    trim tail-drain waits to output-DMA lanes, keeping every compute
    instruction at <=1 sync wait for walrus."""
    import concourse.mybir as mybir

    out_names = set()
    for alloc in nc.m.functions[0].allocations:
        if (
            isinstance(alloc, mybir.MemoryLocationSet)
            and alloc.kind == "ExternalOutput"
        ):
            for ml in alloc.memorylocations:
                out_names.add(ml.name)
    keep_lanes = set()
    for name, inst in nc.inst_map.items():
        if "DMA" not in type(inst).__name__:
            continue
        c = inst.concise()
        if any(f"@{n}" in c.split("in=")[0] for n in out_names):
            for u in (inst.sync_info.on_update or []) if inst.sync_info else []:
                keep_lanes.add(u.ant_name)

    for name, inst in nc.inst_map.items():
        si = inst.sync_info
        if si is None or not si.on_wait or len(si.on_wait) < 2:
            continue
        own = {u.ant_name for u in (si.on_update or [])}
        kept = [w for w in si.on_wait if w.ant_name not in own]
        if "Drain" in type(inst).__name__ and len(kept) > 1:
            kept = [w for w in kept if w.ant_name in keep_lanes]
        if len(kept) != len(si.on_wait):
            si.on_wait = kept


def _prep_shared(W1, b1, Wr, br, W2, b2):
    f32 = np.float32
    W1 = np.asarray(W1, f32); b1 = np.asarray(b1, f32)
    Wr = np.asarray(Wr, f32); br = np.asarray(br, f32)
    W2 = np.asarray(W2, f32); b2 = np.asarray(b2, f32)
    Wrh, Wry = Wr[:, :H], Wr[:, H:]
    Wtil = -0.5 * (Wrh @ W1)                                # [64, 700]
    bt1 = 0.5 * (Wrh @ b1 + br + Wry.sum(axis=1))
    bt2 = 0.5 * (b2 + W2.sum(axis=1))
    # big-mm weights: [chunks, 128(, 2), 64]; feature f = 256c + 128*ko + ki
    Wtp = np.zeros((H, DP), f32)
    Wtp[:, :D] = Wtil
    if USE_DR:
        Wq = np.ascontiguousarray(
            (Wtp * WSCALE).reshape(H, 3, 2, 128).transpose(1, 3, 2, 0)
        ).astype(E4)                                        # [3,128,2,64]
    else:
        Wq = np.ascontiguousarray(
            (Wtp * WSCALE).reshape(H, 6, 128).transpose(2, 1, 0)
        ).astype(E4)                                        # [128,6,64]
    L = np.concatenate([0.5 * Wry.T, 0.5 * W2.T], axis=1).astype(BF16)
    I64 = np.eye(H, dtype=f32).astype(BF16)
    B20 = (1.0 - bt2).reshape(1, C).astype(BF16)
    b64v = (1.0 - bt1).reshape(H, 1).astype(f32)
    return Wq, L, I64, B20, b64v


def _prep_x_core(xbf, k):
    """x slice for core k: steps t in [32k-W, 32k-W+N), zero outside [0,T).

    xbf: (B, T, D) in fp8/bf16 (already cast).  Returns [chunks,128(,2),NCOL].
    """
    t0 = 32 * k - W
    xt = np.zeros((DP, N, BL), xbf.dtype)
    lo = max(0, -t0)
    hi = min(N, T - t0)
    if hi > lo:
        xt[:D, lo:hi] = np.asarray(xbf[:, t0 + lo:t0 + hi, :]).transpose(2, 1, 0)
    xt = xt.reshape(DP, NCOL)
    if USE_DR:
        # feature f = 256c + 128*ko + ki -> [3, 128, 2, NCOL]
        return np.ascontiguousarray(
            xt.reshape(3, 2, 128, NCOL).transpose(0, 2, 1, 3))
    return np.ascontiguousarray(xt.reshape(6, 128, NCOL).transpose(1, 0, 2))


def _ensure_ntff_hook():
    """The RL container's antenv stub lacks axon_hooks; bass_utils imports it
    unconditionally when tracing. Register the ctypes-based hook ourselves."""
    import sys
    import types
    try:
        import antenv
        if "antenv.axon_hooks" in sys.modules:
            return
        mod = types.ModuleType("antenv.axon_hooks")
        _h = [None]
        mod.set_axon_ntff_profile_hook = lambda h: _h.__setitem__(0, h)
        mod.get_axon_ntff_profile_hook = lambda: _h[0]
        sys.modules["antenv.axon_hooks"] = mod
        antenv.axon_hooks = mod
        try:
            from trn_agent_boot.trn_boot import _ntff_profile_via_ctypes
            mod.set_axon_ntff_profile_hook(
                _ntff_profile_via_ctypes("/opt/axon/libaxon_pjrt.so")
            )
        except Exception:
            pass
    except Exception:
        pass


def _combine(res_list):
    """Host combine: sum valid buckets per core -> m2 counts -> output."""
    count = np.zeros((C, BL), np.float64)
    s1count = np.zeros((H, BL), np.float64)
    for k, r in enumerate(res_list):
        S = np.asarray(r["out"], np.float64)        # [84, 3*256]
        lo, md, hi = S[:, 0:BL], S[:, BL:2 * BL], S[:, 2 * BL:3 * BL]
        if k == 0:
            v = md + hi
        elif k == NCORES - 1:
            v = lo + md
        else:
            v = lo + md + hi
        count += v[H:P]
        s1count += v[0:H]
    out = (235.0 - count) / 235.0                   # mean s2, (20, 256)
    return out.astype(np.float32), s1count


def kernel(x, W1, b1, Wr, br, W2, b2):
    from concourse.bass_utils import run_bass_kernel_spmd

    _ensure_ntff_hook()

    if "nc" not in _CACHE:
        _CACHE["nc"] = _build_nc()
    nc = _CACHE["nc"]

    Wq, L, I64, B20, b64v = _prep_shared(W1, b1, Wr, br, W2, b2)

    x = np.asarray(x, np.float32)
    xcast = x.astype(E4)
    in_maps = []
    for k in range(NCORES):
        in_maps.append({
            "xq": _prep_x_core(xcast, k),
            "Wq": Wq, "Lw": L, "I64d": I64, "B20d": B20, "b64": b64v,
        })

    res = run_bass_kernel_spmd(nc, in_maps, core_ids=list(range(NCORES)))
    _CACHE["last_results"] = res
    out, s1count = _combine([r for r in res.results])
    _CACHE["s1count"] = s1count
    return np.ascontiguousarray(out.T)              # (256, 20)
